# revision 1
# baseline (speedup 1.0000x reference)
"""Trainium2 Bass kernel for nn_DenseConcatBlocks (dense_cnn).

Strategy
--------
Data-parallel over batch: 16 images / 8 cores = 2 images per core, weights
replicated, one SPMD NEFF.

Per core, the 9x9 conv of every block is mapped onto the 128x128 PE array
with a Toeplitz "band" scheme that fills both array dimensions despite the
16-channel output:

  - M (stationary cols, 128) = 16 out-channels x 8 consecutive output rows
  - K (partitions, 128)      = 8 in-channels  x 16 input rows (out rows +-4)
  - N (moving dim, 268)      = both images' row pixels side by side (+pad)

The 9 kernel columns (dx) become 9 PSUM-accumulated matmuls reading the
same SBUF band tile at column offsets dx..dx+268; the 9 kernel rows (dy)
are absorbed into the banded stationary operand w1[o, c, g-s, dx].
Channels are chunked 8 at a time (c-chunks) and accumulated as well.

Feature activations live persistently in SBUF in "band layout": one
[128, 280] tile per (c-chunk j, band b); partition c_l*16+g holds channel
8j+c_l, image row 8b-4+g; columns are [8 zeros][img0 row][8 zeros]
[img1 row][8 zeros].  Each block's new channel (sigmoid output) is
scattered into its band slots by small DVE copies (each row lives in two
vertically-overlapping bands).

The 1x1 conv + sigmoid runs as a K=128 (s,o) -> M=8 (s) matmul with a
block-diagonal stationary operand, then two ScalarE sigmoids (bf16 copy
for the next block's features, f32 copy for the DRAM output).

Matmul operands are bf16 (PE streams 1 col/cycle; fp32 would be 4x
slower, fp32r needs rounded producers); PSUM accumulation is fp32 and all
bias/activation math is fp32.
"""

import sys

for _p in ("/opt/trn_rl_repo",):
    if _p not in sys.path:
        sys.path.insert(0, _p)

import numpy as np
import ml_dtypes

import concourse.bass as bass  # noqa: F401  (bass types used via tile/bacc)
import concourse.tile as tile
from concourse import bacc, mybir
from concourse.bass_utils import run_bass_kernel_spmd

H = W = 128
S = 8            # output rows per band
NB = H // S      # 16 bands
G = 16           # input rows per band (S + 8)
CPC = 8          # channels per c-chunk
WS = 280         # band tile width: 8z | 128 img0 | 8z | 128 img1 | 8z
N = 268          # matmul moving dim
IMG0, IMG1 = 8, 144   # storage cols of img0/img1 w=0
P0, P1 = 4, 140       # psum cols of img0/img1 w=0
YW = 276              # y tile width (P0 + 2*136)
NBLK = 50
N_CORES = 8
DT = mybir.dt.bfloat16
NP_DT = ml_dtypes.bfloat16
F32 = mybir.dt.float32
Act = mybir.ActivationFunctionType


def _nch(c_in):
    return (c_in + CPC - 1) // CPC


def _pack_conv_lhsT(w1, nblk):
    """w1 [50,16,51,9,9] f32 -> [T,128,128] bf16 banded stationary tiles.

    Tile t for (blk, chunk j, dx): rows (c_l,g) = c_l*16+g, cols (s,o) =
    s*16+o, value w1[blk, o, 8j+c_l, g-s, dx] when 0 <= g-s <= 8 else 0.
    """
    g_idx, s_idx = np.meshgrid(np.arange(G), np.arange(S), indexing="ij")
    dy = g_idx - s_idx
    valid = (dy >= 0) & (dy <= 8)
    dyc = np.clip(dy, 0, 8)
    tiles = []
    for blk in range(nblk):
        c_in = blk + 1
        nch = _nch(c_in)
        out = np.zeros((nch, 9, 128, 128), np.float32)
        for j in range(nch):
            for c_l in range(CPC):
                c = CPC * j + c_l
                if c >= c_in:
                    continue
                wv = w1[blk, :, c]                      # [16, 9, 9]
                bl = wv[:, dyc, :] * valid[None, :, :, None]   # [16,G,S,9]
                bl = np.transpose(bl, (3, 1, 2, 0))     # [9, G, S, 16]
                out[j, :, c_l * 16:(c_l + 1) * 16, :] = bl.reshape(9, G, 128)
        tiles.append(out.reshape(nch * 9, 128, 128))
    return np.ascontiguousarray(np.concatenate(tiles, 0)).astype(NP_DT)


def _pack_aux(b1, w2, b2, nblk):
    # w1x1: [nblk, 128, 8]; row (s,o), col s' -> w2[blk,o] iff s == s'
    w1x1 = np.zeros((nblk, S, 16, S), np.float32)
    for s in range(S):
        w1x1[:, s, :, s] = w2[:nblk]
    w1x1 = w1x1.reshape(nblk, 128, S).astype(NP_DT)
    b1p = np.ascontiguousarray(np.tile(b1[:nblk], (1, S)), dtype=np.float32)
    b2p = np.ascontiguousarray(
        np.tile(b2[:nblk, None], (1, S)), dtype=np.float32)
    return w1x1, b1p, b2p


def _build(nblk, reps=1):
    nc = bacc.Bacc("TRN2", target_bir_lowering=False, debug=False)
    T = sum(_nch(i + 1) * 9 for i in range(nblk))
    x_in = nc.declare_dram_parameter("x_in", [2, H, W], F32, isOutput=False)
    wconv = nc.declare_dram_parameter("wconv", [T, 128, 128], DT, isOutput=False)
    w1x1 = nc.declare_dram_parameter("w1x1", [nblk, 128, S], DT, isOutput=False)
    b1p = nc.declare_dram_parameter("b1p", [nblk, 128], F32, isOutput=False)
    b2p = nc.declare_dram_parameter("b2p", [nblk, S], F32, isOutput=False)
    out = nc.declare_dram_parameter("out", [2, nblk, H, W], F32, isOutput=True)

    nch_tot = _nch(nblk)

    with tile.TileContext(nc) as tc:
        with (
            tc.tile_pool(name="bands", bufs=1) as bands_pool,
            tc.tile_pool(name="consts", bufs=1) as consts,
            tc.tile_pool(name="wpool", bufs=2) as wpool,
            tc.tile_pool(name="hpool", bufs=3) as hpool,
            tc.tile_pool(name="ypool", bufs=2) as ypool,
            tc.tile_pool(name="pscp", bufs=4, space="PSUM") as pscp,
            tc.tile_pool(name="psyp", bufs=2, space="PSUM") as psyp,
        ):
            bands = [
                [
                    bands_pool.tile(
                        [128, WS], DT, name=f"band_{j}_{b}", tag=f"band_{j}_{b}"
                    )
                    for b in range(NB)
                ]
                for j in range(nch_tot)
            ]
            for j in range(nch_tot):
                for b in range(NB):
                    nc.vector.memset(bands[j][b], 0.0)

            x_sb = consts.tile([128, 2, W], F32, name="x_sb")
            nc.sync.dma_start(out=x_sb, in_=x_in.ap().rearrange("i h w -> h i w"))
            w1x1_sb = consts.tile([128, nblk, S], DT, name="w1x1_sb")
            nc.sync.dma_start(
                out=w1x1_sb, in_=w1x1.ap().rearrange("t p m -> p t m"))
            b1_sb = consts.tile([128, nblk], F32, name="b1_sb")
            nc.sync.dma_start(out=b1_sb, in_=b1p.ap().rearrange("t p -> p t"))
            b2_sb = consts.tile([S, nblk], F32, name="b2_sb")
            nc.sync.dma_start(out=b2_sb, in_=b2p.ap().rearrange("t p -> p t"))

            # x (channel 0) -> bf16, then DMA rows into chunk-0 band tiles.
            # (Engine ops need 32-aligned partition bases; DMA does not.)
            x_bf = consts.tile([128, 2, W], DT, name="x_bf")
            nc.vector.tensor_copy(out=x_bf, in_=x_sb)
            for b in range(NB):
                g0 = 4 if b == 0 else 0
                g1 = 12 if b == NB - 1 else 16
                r0 = 8 * b - 4 + g0
                dst = bands[0][b][g0:g1, IMG0:IMG0 + 272].rearrange(
                    "p (g w) -> p g w", w=136)[:, :, :W]
                nc.sync.dma_start(out=dst, in_=x_bf[r0:r0 + (g1 - g0)])

            for rep in range(reps):
              t0 = 0
              for blk in range(nblk):
                c_in = blk + 1
                nch = _nch(c_in)
                n9 = nch * 9
                w_sb = wpool.tile([128, n9, 128], DT, name=f"w_{blk}", tag="wconv")
                nc.sync.dma_start(
                    out=w_sb,
                    in_=wconv.ap()[t0:t0 + n9].rearrange("t p m -> p t m"))
                t0 += n9
                y_bf = ypool.tile([S, NB, YW], DT, name=f"ybf_{blk}", tag="ybf")
                y_f32 = ypool.tile([S, NB, YW], F32, name=f"yf_{blk}", tag="yf32")
                hs = {}

                def finish(bb, blk=blk, y_bf=y_bf, y_f32=y_f32, hs=hs):
                    psy = psyp.tile(
                        [S, N], F32, name=f"psy_{blk}_{bb}", tag="psy")
                    nc.tensor.matmul(
                        psy, w1x1_sb[:, blk, :], hs.pop(bb),
                        start=True, stop=True)
                    nc.scalar.activation(
                        out=y_bf[:, bb, :N], in_=psy, func=Act.Sigmoid,
                        bias=b2_sb[:, blk:blk + 1], scale=1.0)
                    nc.scalar.activation(
                        out=y_f32[:, bb, :N], in_=psy, func=Act.Sigmoid,
                        bias=b2_sb[:, blk:blk + 1], scale=1.0)

                def scatter(dstb, blk=blk, y_bf=y_bf):
                    c_new = blk + 1
                    base = (c_new % CPC) * 16
                    j2 = c_new // CPC
                    groups = [(4, 8, dstb, 0)]
                    if dstb + 1 < NB:
                        groups.append((12, 4, dstb + 1, 0))
                    if dstb - 1 >= 0:
                        groups.append((0, 4, dstb - 1, 4))
                    for (g0, n, sb_, s0) in groups:
                        src = y_bf[s0:s0 + n, sb_, P0:P0 + 272].rearrange(
                            "s (g w) -> s g w", w=136)[:, :, :W]
                        dst = bands[j2][dstb][
                            base + g0:base + g0 + n, IMG0:IMG0 + 272
                        ].rearrange("p (g w) -> p g w", w=136)[:, :, :W]
                        nc.sync.dma_start(out=dst, in_=src)

                for b in range(NB):
                    psc = pscp.tile(
                        [128, N], F32, name=f"psc_{blk}_{b}", tag="psc")
                    k = 0
                    for j in range(nch):
                        for dx in range(9):
                            nc.tensor.matmul(
                                psc,
                                w_sb[:, j * 9 + dx, :],
                                bands[j][b][:, dx:dx + N],
                                start=(k == 0),
                                stop=(k == n9 - 1))
                            k += 1
                    h_t = hpool.tile([128, N], DT, name=f"h_{blk}_{b}", tag="h")
                    nc.scalar.activation(
                        out=h_t, in_=psc, func=Act.Relu,
                        bias=b1_sb[:, blk:blk + 1], scale=1.0)
                    hs[b] = h_t
                    if b >= 1:
                        finish(b - 1)
                    if b >= 2 and blk + 1 < nblk:
                        scatter(b - 2)
                finish(NB - 1)
                if blk + 1 < nblk:
                    scatter(NB - 2)
                    scatter(NB - 1)
                for img, pc in ((0, P0), (1, P1)):
                    dst = out.ap()[img, blk].rearrange("(b s) w -> s b w", s=S)
                    nc.sync.dma_start(
                        out=dst, in_=y_f32[:, :, pc:pc + W])

    nc.compile()
    return nc


def _run(x, w1, b1, w2, b2, nblk=NBLK, trace=False):
    x = np.asarray(x, np.float32)
    wconv_np = _pack_conv_lhsT(np.asarray(w1, np.float32), nblk)
    w1x1_np, b1p_np, b2p_np = _pack_aux(
        np.asarray(b1, np.float32), np.asarray(w2, np.float32),
        np.asarray(b2, np.float32), nblk)
    nc = _build(nblk)
    in_maps = []
    for k in range(N_CORES):
        in_maps.append({
            "x_in": np.ascontiguousarray(x[2 * k:2 * k + 2, 0]),
            "wconv": wconv_np,
            "w1x1": w1x1_np,
            "b1p": b1p_np,
            "b2p": b2p_np,
        })
    res = run_bass_kernel_spmd(nc, in_maps, list(range(N_CORES)), trace=trace)
    full = np.concatenate([res.results[k]["out"] for k in range(N_CORES)], 0)
    return full, res


def kernel(**inputs):
    full, _ = _run(
        inputs["x"], inputs["w1"], inputs["b1"], inputs["w2"], inputs["b2"])
    return full.astype(np.float32)



# revision 7
# speedup vs baseline: 1.3758x; 1.3758x over previous
"""Trainium2 Bass kernel for nn_DenseConcatBlocks (dense_cnn).

Strategy
--------
Data-parallel over batch: 16 images / 8 cores = 2 images per core, weights
replicated, one SPMD NEFF.

Per core, the 9x9 conv of every block is mapped onto the 128x128 PE array
with a Toeplitz "band" scheme that fills both array dimensions despite the
16-channel output:

  - M (stationary cols, 128) = 16 out-channels x 8 consecutive output rows
  - K (partitions, 128)      = 8 in-channels  x 16 input rows (out rows +-4)
  - N (moving dim, 264)      = both images' row pixels side by side (+pad)

The 9 kernel columns (dx) become PSUM-accumulated matmuls reading the band
storage at column offsets; the 9 kernel rows (dy) are absorbed into the
banded stationary operand w1[o, c, g-s, dx].

fp8 DoubleRow: conv matmuls run in fp8e4m3 with MatmulPerfMode.DoubleRow
(0.5 cycles/row = 2x bf16), processing TWO adjacent 8-channel chunks per
matmul via a [128, 2, N] moving AP with middle stride WS (non-overlapping
windows; overlapping strides are rejected by hardware).  Odd chunk counts
are padded with a zero-weight chunk.  All bands and chunks live in ONE
wide SBUF tile [128, NB*WIDE_W] so cross-chunk pairs are addressable and
the per-block feature scatter batches into 6 multi-band DMAs (instead of
~46 tiny ones, which serialized the DMA queue).

fp8 precision is protected three ways:
  1. Activations are stored CENTERED: tanh(z/2) = 2*(sigmoid(z)-0.5), so
     quantization error multiplies the small centered part, not the 0.5
     mean.  The exact mean contribution 0.5*sum(w) folds into the conv
     bias host-side (padding stores -1.0 == "activation 0" to keep the
     correction spatially uniform).
  2. Weights are pre-scaled x64 (x32 for centered channels) to clear the
     fp8 subnormal range; the ReLU activation descales via its `scale`.
  3. The 1x1 conv + h stay bf16.

Engine split: PE matmuls; ScalarE relu + sigmoid(f32); DVE derives the
fp8 feature copy as 2*sigmoid-1; SP issues all DMAs.
"""

import sys

for _p in ("/opt/trn_rl_repo",):
    if _p not in sys.path:
        sys.path.insert(0, _p)

import numpy as np
import ml_dtypes

import concourse.bass as bass  # noqa: F401  (bass types used via tile/bacc)
import concourse.tile as tile
from concourse import bacc, mybir
from concourse.bass_utils import run_bass_kernel_spmd

H = W = 128
S = 8            # output rows per band
NB = H // S      # 16 bands
G = 16           # input rows per band (S + 8)
CPC = 8          # channels per c-chunk
WS = 272         # chunk width: 4z | 128 img0 | 8z | 128 img1 | 4z
N = 264          # matmul moving dim
NCH_PAD = 8      # chunk regions per band (ceil(50/8) padded to even)
WIDE_W = NCH_PAD * WS + 4   # +4 so the 272-col scatter view fits in chunk 7
IMG0, IMG1 = 4, 140   # storage cols of img0/img1 x=0 (within a chunk)
P0, P1 = 0, 136       # psum cols of img0/img1 x=0
YW = 272              # y tile width (272-col rearrange view)
NBLK = 50
N_CORES = 8
WSC = 64.0       # weight scale (fp8 subnormal avoidance)
USE_DVE_DERIVE = True
DT = mybir.dt.bfloat16
F8 = mybir.dt.float8e4
NP_F8 = ml_dtypes.float8_e4m3
NP_DT = ml_dtypes.bfloat16
F32 = mybir.dt.float32
Act = mybir.ActivationFunctionType
Alu = mybir.AluOpType
DR = mybir.MatmulPerfMode.DoubleRow


def _nch(c_in):
    return (c_in + CPC - 1) // CPC


def _npc(c_in):
    n = _nch(c_in)
    return n + (n & 1)


def _pack_conv_lhsT(w1, nblk):
    """w1 [50,16,51,9,9] f32 -> [T,128,128] fp8e4m3 banded stationary tiles.

    Per block: chunk count padded to even npc; tiles ordered (u, dx, i)
    where u = chunk-pair index and i in {0,1} selects chunk 2u+i, so the
    DoubleRow stationary slice [:, t:t+2, :] holds one (pair, dx).
    Tile value at row (c_l,g) = c_l*16+g, col (s,o) = s*16+o:
    w1[blk, o, 8*(2u+i)+c_l, g-s, dx] * scale for 0 <= g-s <= 8 else 0.
    scale = WSC for channel 0 (x, stored as-is), WSC/2 for channels >= 1
    (stored as tanh(z/2) = 2*(sigmoid-0.5)).
    """
    g_idx, s_idx = np.meshgrid(np.arange(G), np.arange(S), indexing="ij")
    dy = g_idx - s_idx
    valid = (dy >= 0) & (dy <= 8)
    dyc = np.clip(dy, 0, 8)
    tiles = []
    for blk in range(nblk):
        c_in = blk + 1
        npc = _npc(c_in)
        chunk = np.zeros((npc, 9, 128, 128), np.float32)
        for j in range(npc):
            for c_l in range(CPC):
                c = CPC * j + c_l
                if c >= c_in:
                    continue
                sc = WSC if c == 0 else WSC * 0.5
                wv = w1[blk, :, c] * sc                 # [16, 9, 9]
                bl = wv[:, dyc, :] * valid[None, :, :, None]   # [16,G,S,9]
                bl = np.transpose(bl, (3, 1, 2, 0))     # [9, G, S, 16]
                chunk[j, :, c_l * 16:(c_l + 1) * 16, :] = bl.reshape(9, G, 128)
        # (j, dx, ...) -> (u, dx, i, ...)
        ord_ = chunk.reshape(npc // 2, 2, 9, 128, 128).transpose(0, 2, 1, 3, 4)
        tiles.append(np.ascontiguousarray(ord_).reshape(npc * 9, 128, 128))
    return np.ascontiguousarray(np.concatenate(tiles, 0)).astype(NP_F8)


def _pack_aux(w1, b1, w2, b2, nblk):
    # w1x1: [nblk, 128, 8]; row (s,o), col s' -> w2[blk,o] iff s == s'
    w1x1 = np.zeros((nblk, S, 16, S), np.float32)
    for s in range(S):
        w1x1[:, s, :, s] = w2[:nblk]
    w1x1 = w1x1.reshape(nblk, 128, S).astype(NP_DT)
    # conv bias + centered-activation correction: stored features hold
    # a - 0.5 (scaled), incl. -0.5 in padding, so psum/WSC =
    # conv_true - 0.5 * K_o with K_o = sum over y-channel taps of w1.
    ko = np.zeros((nblk, 16), np.float32)
    for blk in range(1, nblk):
        ko[blk] = w1[blk, :, 1:blk + 1].sum(axis=(1, 2, 3))
    b1c = (b1[:nblk] + 0.5 * ko).astype(np.float32)
    b1p = np.ascontiguousarray(np.tile(b1c, (1, S)), dtype=np.float32)
    b2p = np.ascontiguousarray(
        np.tile(b2[:nblk, None], (1, S)), dtype=np.float32)
    return w1x1, b1p, b2p


def _build(nblk, reps=1):
    nc = bacc.Bacc("TRN2", target_bir_lowering=False, debug=False)
    T = sum(_npc(i + 1) * 9 for i in range(nblk))
    x_in = nc.declare_dram_parameter("x_in", [2, H, W], F32, isOutput=False)
    wconv = nc.declare_dram_parameter("wconv", [T, 128, 128], F8, isOutput=False)
    w1x1 = nc.declare_dram_parameter("w1x1", [nblk, 128, S], DT, isOutput=False)
    b1p = nc.declare_dram_parameter("b1p", [nblk, 128], F32, isOutput=False)
    b2p = nc.declare_dram_parameter("b2p", [nblk, S], F32, isOutput=False)
    out = nc.declare_dram_parameter("out", [2, nblk, H, W], F32, isOutput=True)

    with tile.TileContext(nc) as tc:
        with (
            tc.tile_pool(name="bands", bufs=1) as bands_pool,
            tc.tile_pool(name="consts", bufs=1) as consts,
            tc.tile_pool(name="wpool", bufs=2) as wpool,
            tc.tile_pool(name="hpool", bufs=3) as hpool,
            tc.tile_pool(name="ypool", bufs=2) as ypool,
            tc.tile_pool(name="pscp", bufs=4, space="PSUM") as pscp,
            tc.tile_pool(name="psyp", bufs=2, space="PSUM") as psyp,
        ):
            # one mega-tile: band b chunk j at cols b*WIDE_W + j*WS
            bandsT = bands_pool.tile(
                [128, NB * WIDE_W], F8, name="bands", tag="bands")
            # -1.0 == stored "activation 0" for centered channels;
            # x rows (chunk 0, partitions 0:16) pad with 0 instead.
            nc.vector.memset(bandsT, -1.0)
            nc.vector.memset(
                bandsT[0:16, :].rearrange(
                    "p (b r) -> p b r", r=WIDE_W)[:, :, 0:WS], 0.0)

            x_sb = consts.tile([128, 2, W], F32, name="x_sb")
            nc.sync.dma_start(out=x_sb, in_=x_in.ap().rearrange("i h w -> h i w"))
            w1x1_sb = consts.tile([128, nblk, S], DT, name="w1x1_sb")
            nc.sync.dma_start(
                out=w1x1_sb, in_=w1x1.ap().rearrange("t p m -> p t m"))
            b1_sb = consts.tile([128, nblk], F32, name="b1_sb")
            nc.sync.dma_start(out=b1_sb, in_=b1p.ap().rearrange("t p -> p t"))
            b2_sb = consts.tile([S, nblk], F32, name="b2_sb")
            nc.sync.dma_start(out=b2_sb, in_=b2p.ap().rearrange("t p -> p t"))
            b2h_sb = consts.tile([S, nblk], F32, name="b2h_sb")
            nc.vector.tensor_scalar_mul(out=b2h_sb, in0=b2_sb, scalar1=0.5)

            # x (channel 0) -> fp8, then DMA rows into chunk-0 band slots.
            # (Engine ops need 32-aligned partition bases; DMA does not.)
            x_f8 = consts.tile([128, 2, W], F8, name="x_f8")
            nc.vector.tensor_copy(out=x_f8, in_=x_sb)
            for b in range(NB):
                g0 = 4 if b == 0 else 0
                g1 = 12 if b == NB - 1 else 16
                r0 = 8 * b - 4 + g0
                dst = bandsT[
                    g0:g1, b * WIDE_W + IMG0:b * WIDE_W + IMG0 + 272
                ].rearrange("p (g w) -> p g w", w=136)[:, :, :W]
                nc.sync.dma_start(out=dst, in_=x_f8[r0:r0 + (g1 - g0)])

            def mv_pair(b, u, dx):
                ap = bandsT[:, 0:2 * N].rearrange("p (a c) -> p a c", a=2)
                v = ap.ap
                v[1] = (WS, 2)
                ap.ap = v
                ap.offset = ap.offset + b * WIDE_W + 2 * u * WS + dx
                return ap

            for rep in range(reps):
              t0 = 0
              for blk in range(nblk):
                c_in = blk + 1
                npc = _npc(c_in)
                nu = npc // 2
                n9 = npc * 9
                w_sb = wpool.tile([128, n9, 128], F8, name=f"w_{blk}", tag="wconv")
                nc.sync.dma_start(
                    out=w_sb,
                    in_=wconv.ap()[t0:t0 + n9].rearrange("t p m -> p t m"))
                t0 += n9
                y_c8 = ypool.tile([S, NB, YW], F8, name=f"yc_{blk}", tag="yc8")
                y_f32 = ypool.tile([S, NB, YW], F32, name=f"yf_{blk}", tag="yf32")
                hs = {}

                def finish(bb, blk=blk, y_c8=y_c8, y_f32=y_f32, hs=hs):
                    psy = psyp.tile(
                        [S, N], F32, name=f"psy_{blk}_{bb}", tag="psy")
                    nc.tensor.matmul(
                        psy, w1x1_sb[:, blk, :], hs.pop(bb),
                        start=True, stop=True)
                    nc.scalar.activation(
                        out=y_f32[:, bb, :N], in_=psy, func=Act.Sigmoid,
                        bias=b2_sb[:, blk:blk + 1], scale=1.0)
                    # fp8 feature copy = 2*sigmoid - 1 = tanh(z/2), on DVE
                    if USE_DVE_DERIVE:
                        nc.vector.tensor_scalar(
                            out=y_c8[:, bb, :N], in0=y_f32[:, bb, :N],
                            scalar1=2.0, scalar2=-1.0,
                            op0=Alu.mult, op1=Alu.add)
                    else:
                        nc.scalar.activation(
                            out=y_c8[:, bb, :N], in_=psy, func=Act.Tanh,
                            bias=b2h_sb[:, blk:blk + 1], scale=0.5)

                def scatter(half, blk=blk, y_c8=y_c8):
                    c_new = blk + 1
                    base = (c_new % CPC) * 16
                    jcol = (c_new // CPC) * WS
                    b0 = 8 * half
                    # (g0, gn, src_s0, src_b0, dst_b0, nb); dst band db
                    # g rows 12..16 hold y-band db+1 rows 0..4, and
                    # g rows 0..4 hold y-band db-1 rows 4..8
                    groups = [(4, 8, 0, b0, b0, 8)]          # center
                    if half == 0:
                        groups.append((12, 4, 0, 1, 0, 8))   # dst db <- src db+1
                        groups.append((0, 4, 4, 0, 1, 8))    # dst db <- src db-1
                    else:
                        groups.append((12, 4, 0, 9, 8, 7))
                        groups.append((0, 4, 4, 8, 9, 7))
                    for (g0, gn, s0, sb0, db0, nb) in groups:
                        # one DMA per image: DMA lowering caps balanced
                        # APs at 3 dims
                        for ic in (0, 136):
                            src = y_c8[
                                s0:s0 + gn, sb0:sb0 + nb, ic:ic + W]
                            dst = bandsT[
                                base + g0:base + g0 + gn,
                                db0 * WIDE_W:(db0 + nb) * WIDE_W,
                            ].rearrange("p (b r) -> p b r", r=WIDE_W)[
                                :, :, jcol + IMG0 + ic:jcol + IMG0 + ic + W]
                            nc.sync.dma_start(out=dst, in_=src)

                for b in range(NB):
                    psc = pscp.tile(
                        [128, N], F32, name=f"psc_{blk}_{b}", tag="psc")
                    k, klast = 0, nu * 9 - 1
                    for u in range(nu):
                        for dx in range(9):
                            nc.tensor.matmul(
                                psc,
                                w_sb[:, 2 * (u * 9 + dx):2 * (u * 9 + dx) + 2, :],
                                mv_pair(b, u, dx),
                                start=(k == 0),
                                stop=(k == klast),
                                perf_mode=DR)
                            k += 1
                    h_t = hpool.tile([128, N], DT, name=f"h_{blk}_{b}", tag="h")
                    nc.scalar.activation(
                        out=h_t, in_=psc, func=Act.Relu,
                        bias=b1_sb[:, blk:blk + 1], scale=1.0 / WSC)
                    hs[b] = h_t
                    if b >= 1:
                        finish(b - 1)
                    if b == 9 and blk + 1 < nblk:
                        scatter(0)
                finish(NB - 1)
                if blk + 1 < nblk:
                    scatter(1)
                for img, pc in ((0, P0), (1, P1)):
                    dst = out.ap()[img, blk].rearrange("(b s) w -> s b w", s=S)
                    nc.sync.dma_start(
                        out=dst, in_=y_f32[:, :, pc:pc + W])

    nc.compile()
    return nc


def _run(x, w1, b1, w2, b2, nblk=NBLK, trace=False):
    x = np.asarray(x, np.float32)
    wconv_np = _pack_conv_lhsT(np.asarray(w1, np.float32), nblk)
    w1x1_np, b1p_np, b2p_np = _pack_aux(
        np.asarray(w1, np.float32), np.asarray(b1, np.float32),
        np.asarray(w2, np.float32), np.asarray(b2, np.float32), nblk)
    nc = _build(nblk)
    in_maps = []
    for k in range(N_CORES):
        in_maps.append({
            "x_in": np.ascontiguousarray(x[2 * k:2 * k + 2, 0]),
            "wconv": wconv_np,
            "w1x1": w1x1_np,
            "b1p": b1p_np,
            "b2p": b2p_np,
        })
    res = run_bass_kernel_spmd(nc, in_maps, list(range(N_CORES)), trace=trace)
    full = np.concatenate([res.results[k]["out"] for k in range(N_CORES)], 0)
    return full, res


def kernel(**inputs):
    full, _ = _run(
        inputs["x"], inputs["w1"], inputs["b1"], inputs["w2"], inputs["b2"])
    return full.astype(np.float32)
